# revision 7
# baseline (speedup 1.0000x reference)
"""GCNConv (gnn_message_passing) on 8 Trainium2 NeuronCores — v6.

out = D^{-1/2} (A + I) D^{-1/2} (X W) + b

Host folds dinv into x rows (prescale), so the device computes
  h = (dinv*x) @ W          (f32 table, replicated on every core)
  out[d] = dinv[d] * sum_{e: dst=d} h[src_e]     (self loops are edges)

Pipeline (v6): the h table lives in 4 per-bank DRAM tiles (32768 rows
each — the int16 reach of dma_gather).  Edge gathers are batched per
(bank, superwindow of SW dst-windows) into gpsimd.dma_gather ops
(~4.7k rows each, single_packet=False, 4 SWDGE queues round-robin);
bank-major order lets bank-b gathers start as soon as phase A has
written bank b, overlapping the dense phase under the gather stream.
Per-window partial sums accumulate across banks in an SBUF f32
accumulator: psum[128dst, 64] += seg^T @ msg per (window, bank), then
acc[:, w, :] += psum (DVE).  elem_size_bytes must be %256 -> f32 rows;
matmul rhs reads a stride-2 bf16 bitcast view (bf16 == high half of
f32), so no cast pass.  seg one-hots come from one is_equal per
(window, bank) on doff (laid out (bank, window)-major).  Final:
out[w] = acc * dinv, stored f32 node-major; host concat + bias.
"""

import numpy as np

P = 128
COUT = 64
CH = 4096          # phase A node chunk
NQ = 4             # SWDGE queues
BANK = 32768       # dma_gather int16 index reach (rows per bank)
SW = 4             # windows per superwindow (gather batch)


def _cdiv(a, b):
    return -(-a // b)


# ----------------------------------------------------------------------------
# CPU planning
# ----------------------------------------------------------------------------
def _plan(edge_index, N, ncores):
    shard = N // ncores
    nwin = _cdiv(shard, P)
    npad = _cdiv(N, CH) * CH if N % CH else N + CH   # h table rows
    nbank = _cdiv(npad, BANK)
    nsw = _cdiv(nwin, SW)

    src = np.asarray(edge_index[0], dtype=np.int64)
    dst = np.asarray(edge_index[1], dtype=np.int64)
    deg = np.bincount(dst, minlength=N).astype(np.float64) + 1.0
    dinv = (1.0 / np.sqrt(deg)).astype(np.float32)

    loop = np.arange(N, dtype=np.int64)
    src = np.concatenate([src, loop])
    dst = np.concatenate([dst, loop])

    per_core = []
    cnt = np.zeros((ncores, nwin, nbank), np.int64)
    for c in range(ncores):
        m = (dst // shard) == c
        s = src[m]
        d = dst[m] - c * shard
        w = d >> 7
        b = s // BANK
        order = np.lexsort((w, b))
        s, d, w, b = s[order], d[order], w[order], b[order]
        np.add.at(cnt[c], (w, b), 1)
        per_core.append((s, d, w, b))

    G = _cdiv(cnt.max(axis=0), P)            # [nwin, nbank] groups (uniform)

    # global group (slot/msg) order: (bank, sw, window)
    # doff column order:            (bank, window)
    gslot = np.zeros((nwin, nbank), np.int64)   # group base in slot order
    gdoff = np.zeros((nwin, nbank), np.int64)   # doff col base of (w, b) run
    gops = []    # per (b, si): dict(g0, ng, ws=[w...])
    gpos = 0
    for b in range(nbank):
        for si in range(nsw):
            ws = list(range(si * SW, min((si + 1) * SW, nwin)))
            g0 = gpos
            for w in ws:
                gslot[w, b] = gpos
                gpos += G[w, b]
            if gpos > g0:
                gops.append(dict(b=b, g0=g0, ng=gpos - g0, ws=ws))
    GT = gpos
    dpos = 0
    for b in range(nbank):
        for w in range(nwin):
            gdoff[w, b] = dpos
            dpos += G[w, b]
    assert dpos == GT
    slots = GT * P

    idx16 = np.zeros((ncores, 128, slots // 16), np.int16)
    dstoff = np.empty((ncores, P, GT), np.float32)
    for c in range(ncores):
        s, d, w, b = per_core[c]
        gidx = np.zeros(slots, np.int64)         # bank-relative row (pad -> 0)
        wb = b * nwin + w
        runstart = np.concatenate([[0], 1 + np.flatnonzero(wb[1:] != wb[:-1])])
        rank = np.arange(len(wb)) - np.repeat(
            runstart, np.diff(np.concatenate([runstart, [len(wb)]])))
        slot = (gslot[w, b] + (rank >> 7)) * P + (rank & 127)
        gidx[slot] = s - b * BANK
        dslot = (gdoff[w, b] + (rank >> 7)) * P + (rank & 127)
        dofftmp = np.full(slots, -1.0, np.float32)
        dofftmp[dslot] = (d & 127).astype(np.float32)
        dstoff[c] = dofftmp.reshape(GT, P).T
        a = gidx.astype(np.int16).reshape(slots // 16, 16).T
        idx16[c] = np.tile(a, (8, 1))

    return dict(shard=shard, nwin=nwin, npad=npad, nbank=nbank, nsw=nsw,
                G=G, GT=GT, slots=slots, gops=gops,
                gslot=gslot, gdoff=gdoff,
                dinv=dinv, idx16=idx16, dstoff=dstoff)


# ----------------------------------------------------------------------------
# Device program
# ----------------------------------------------------------------------------
def _build(plan, N, CIN, ncores, unroll=1, skip=()):
    import concourse.bacc as bacc
    import concourse.tile as tile
    import concourse.mybir as mybir

    f32 = mybir.dt.float32
    bf16 = mybir.dt.bfloat16
    i16 = mybir.dt.int16
    nwin, npad, nbank = plan["nwin"], plan["npad"], plan["nbank"]
    G, GT, slots = plan["G"], plan["GT"], plan["slots"]
    gops, gslot, gdoff = plan["gops"], plan["gslot"], plan["gdoff"]
    padn = nwin * P
    kblk = CIN // P
    opGmax = max(op["ng"] for op in gops)
    wbGmax = int(G.max())

    nc = bacc.Bacc("TRN2", target_bir_lowering=False, debug=False,
                   enable_asserts=False, num_devices=ncores,
                   num_swdge_queues=NQ)

    xsT_in = nc.dram_tensor("xsT", [CIN, npad], bf16, kind="ExternalInput")
    w_in = nc.dram_tensor("w2", [P, kblk * COUT], bf16, kind="ExternalInput")
    idx_in = nc.dram_tensor("gidx", [P, slots // 16], i16,
                            kind="ExternalInput")
    doff_in = nc.dram_tensor("dstoff", [P, GT], bf16, kind="ExternalInput")
    iota_in = nc.dram_tensor("iota", [P, P], bf16, kind="ExternalInput")
    dinv_in = nc.dram_tensor("dinv_t", [P, nwin], f32, kind="ExternalInput")
    out_t = nc.dram_tensor("out", [padn, COUT], f32, kind="ExternalOutput")

    qi = 0

    with tile.TileContext(nc) as tc:
        with (
            tc.tile_pool(name="dram", bufs=1, space="DRAM") as dram,
            tc.tile_pool(name="const", bufs=1) as const,
            tc.tile_pool(name="acc", bufs=1) as accp,
            tc.tile_pool(name="xp", bufs=2) as xp,
            tc.tile_pool(name="hsb", bufs=3) as hsp,
            tc.tile_pool(name="idx", bufs=3) as idxp,
            tc.tile_pool(name="msgf", bufs=3) as msgfp,
            tc.tile_pool(name="seg", bufs=4) as segp,
            tc.tile_pool(name="osb", bufs=4) as osbp,
            tc.tile_pool(name="psA", bufs=2, space="PSUM") as psA,
            tc.tile_pool(name="psB", bufs=4, space="PSUM") as psB,
        ):
            h2b = [dram.tile([min(BANK, npad - b * BANK), COUT], f32,
                             name=f"h2b{b}")
                   for b in range(nbank)]

            for _it in range(unroll):
                w_sb = const.tile([P, kblk * COUT], bf16, tag="w_sb")
                nc.sync.dma_start(w_sb[:], w_in[:, :])
                iota_sb = const.tile([P, P], bf16, tag="iota")
                nc.sync.dma_start(iota_sb[:], iota_in[:, :])
                doff_sb = const.tile([P, GT], bf16, tag="doff")
                nc.sync.dma_start(doff_sb[:], doff_in[:, :])
                dinv_sb = const.tile([P, nwin], f32, tag="dinv")
                nc.sync.dma_start(dinv_sb[:], dinv_in[:, :])

                acc = accp.tile([P, nwin, COUT], f32, tag="acc")
                nc.vector.memset(acc[:], 0.0)

                # ---------------- Phase A: h = xs @ W (replicated, f32) ----
                for c0 in range(0, npad, CH):
                    xt = xp.tile([P, kblk, CH], bf16, tag="xt")
                    nc.sync.dma_start(
                        xt[:],
                        xsT_in[:, c0:c0 + CH].rearrange(
                            "(k p) n -> p k n", p=P))
                    hb = h2b[c0 // BANK]
                    r0 = c0 % BANK
                    for s in range(CH // 1024):
                        ps = psA.tile([P, 8, COUT], f32)
                        for t in range(8):
                            base = s * 1024 + t * P
                            for k in range(kblk):
                                nc.tensor.matmul(
                                    out=ps[:, t, :],
                                    lhsT=xt[:, k, base:base + P],
                                    rhs=w_sb[:, k * COUT:(k + 1) * COUT],
                                    start=(k == 0), stop=(k == kblk - 1))
                        hsb = hsp.tile([P, 8, COUT], f32, tag="hsb")
                        nc.scalar.copy(hsb[:], ps[:])
                        nc.sync.dma_start(
                            hb[r0 + s * 1024:r0 + (s + 1) * 1024, :]
                            .rearrange("(b p) q -> p b q", p=P),
                            hsb[:])

                # ---------------- Phase B: batched gather + one-hot matmul -
                if "phaseB" in skip:
                    continue
                for op in gops:
                    b, g0, ng, ws = op["b"], op["g0"], op["ng"], op["ws"]
                    idx_sb = idxp.tile([P, opGmax * 8], i16, tag="idx")
                    nc.sync.dma_start(
                        idx_sb[:, :ng * 8],
                        idx_in[:, g0 * 8:(g0 + ng) * 8])
                    msgf = msgfp.tile([P, opGmax, COUT], f32, tag="msgf")
                    nc.gpsimd.dma_gather(
                        out_ap=msgf[:, :ng, :],
                        in_ap=h2b[b][:, :],
                        idxs_ap=idx_sb[:, :ng * 8],
                        num_idxs=ng * P,
                        num_idxs_reg=ng * P,
                        elem_size=COUT,
                        queue_num=qi % NQ,
                        single_packet=False,
                    )
                    qi += 1
                    for w in ws:
                        gw = int(G[w, b])
                        if gw == 0:
                            continue
                        d0 = int(gdoff[w, b])
                        m0 = int(gslot[w, b]) - g0
                        seg = segp.tile([P, wbGmax, P], bf16, tag="seg")
                        nc.vector.tensor_tensor(
                            out=seg[:, :gw, :],
                            in0=doff_sb[:, d0:d0 + gw, None]
                                .to_broadcast([P, gw, P]),
                            in1=iota_sb[:, None, :].to_broadcast([P, gw, P]),
                            op=mybir.AluOpType.is_equal)
                        ps = psB.tile([P, COUT], f32)
                        for j in range(gw):
                            rhs = (msgf[:, m0 + j, :]
                                   .bitcast(bf16)[:, 1::2])
                            nc.tensor.matmul(
                                out=ps[:], lhsT=seg[:, j, :], rhs=rhs,
                                start=(j == 0), stop=(j == gw - 1))
                        nc.vector.tensor_tensor(
                            out=acc[:, w, :], in0=ps[:], in1=acc[:, w, :],
                            op=mybir.AluOpType.add)

                for w in range(nwin):
                    o_sb = osbp.tile([P, COUT], f32, tag="osb")
                    nc.vector.tensor_scalar_mul(o_sb[:], acc[:, w, :],
                                                dinv_sb[:, w:w + 1])
                    nc.sync.dma_start(out_t[w * P:(w + 1) * P, :], o_sb[:])

    nc.compile()
    return nc


# ----------------------------------------------------------------------------
# Entry point
# ----------------------------------------------------------------------------
def _prepare(x, edge_index, W, b, ncores=8):
    from concourse import mybir

    bf16 = mybir.dt.np(mybir.dt.bfloat16)
    x = np.asarray(x)
    W = np.asarray(W)
    N, CIN = x.shape
    plan = _plan(edge_index, N, ncores)
    shard, nwin, npad = plan["shard"], plan["nwin"], plan["npad"]
    dinv = plan["dinv"]

    xsT = np.zeros((CIN, npad), dtype=bf16)
    xsT[:, :N] = (x.astype(np.float32) * dinv[:, None]).T.astype(bf16)
    w2 = np.concatenate([W[:P, :], W[P:, :]], axis=1).astype(bf16)
    iota = np.tile(np.arange(P, dtype=np.float32), (P, 1)).astype(bf16)

    in_maps = []
    for c in range(ncores):
        dv = np.zeros((nwin * P,), np.float32)
        dv[:shard] = dinv[c * shard:(c + 1) * shard]
        in_maps.append({
            "xsT": xsT,
            "w2": w2,
            "gidx": np.ascontiguousarray(plan["idx16"][c]),
            "dstoff": plan["dstoff"][c].astype(bf16),
            "iota": iota,
            "dinv_t": np.ascontiguousarray(dv.reshape(nwin, P).T),
        })
    return plan, in_maps


def kernel(x, edge_index, W, b, _trace=False):
    from concourse.bass_utils import run_bass_kernel_spmd

    x = np.asarray(x)
    W = np.asarray(W)
    b = np.asarray(b)
    N, CIN = x.shape
    ncores = 8
    plan, in_maps = _prepare(x, edge_index, W, b, ncores)
    shard = plan["shard"]

    nc = _build(plan, N, CIN, ncores)

    kernel.last_build = lambda unroll: (
        nc if unroll == 1 else _build(plan, N, CIN, ncores, unroll=unroll))
    kernel.last_in_maps = in_maps
    res = run_bass_kernel_spmd(nc, in_maps, core_ids=list(range(ncores)))
    out = np.concatenate(
        [r["out"][:shard].astype(np.float32) for r in res.results], axis=0)
    out = out + b.astype(np.float32)
    kernel.last_results = res
    return out


# revision 10
# speedup vs baseline: 1.1939x; 1.1939x over previous
"""GCNConv (gnn_message_passing) on 8 Trainium2 NeuronCores — v6.

out = D^{-1/2} (A + I) D^{-1/2} (X W) + b

Host folds dinv into x rows (prescale), so the device computes
  h = (dinv*x) @ W          (f32 table, replicated on every core)
  out[d] = dinv[d] * sum_{e: dst=d} h[src_e]     (self loops are edges)

Pipeline (v6): the h table lives in 4 per-bank DRAM tiles (32768 rows
each — the int16 reach of dma_gather).  Edge gathers are batched per
(bank, superwindow of SW dst-windows) into gpsimd.dma_gather ops
(~4.7k rows each, single_packet=False, 4 SWDGE queues round-robin);
bank-major order lets bank-b gathers start as soon as phase A has
written bank b, overlapping the dense phase under the gather stream.
Per-window partial sums accumulate across banks in an SBUF f32
accumulator: psum[128dst, 64] += seg^T @ msg per (window, bank), then
acc[:, w, :] += psum (DVE).  elem_size_bytes must be %256 -> f32 rows;
matmul rhs reads a stride-2 bf16 bitcast view (bf16 == high half of
f32), so no cast pass.  seg one-hots come from one is_equal per
(window, bank) on doff (laid out (bank, window)-major).  Final:
out[w] = acc * dinv, stored f32 node-major; host concat + bias.
"""

import numpy as np

P = 128
COUT = 64
CH = 4096          # phase A node chunk
NQ = 4             # SWDGE queues
BANK = 32768       # dma_gather int16 index reach (rows per bank)
SW = 4             # windows per superwindow (gather batch)


def _cdiv(a, b):
    return -(-a // b)


# ----------------------------------------------------------------------------
# CPU planning
# ----------------------------------------------------------------------------
def _plan(edge_index, N, ncores):
    shard = N // ncores
    nwin = _cdiv(shard, P)
    npad = _cdiv(N, CH) * CH if N % CH else N + CH   # h table rows
    nbank = _cdiv(npad, BANK)
    nsw = _cdiv(nwin, SW)

    src = np.asarray(edge_index[0], dtype=np.int64)
    dst = np.asarray(edge_index[1], dtype=np.int64)
    deg = np.bincount(dst, minlength=N).astype(np.float64) + 1.0
    dinv = (1.0 / np.sqrt(deg)).astype(np.float32)

    loop = np.arange(N, dtype=np.int64)
    src = np.concatenate([src, loop])
    dst = np.concatenate([dst, loop])

    per_core = []
    cnt = np.zeros((ncores, nwin, nbank), np.int64)
    for c in range(ncores):
        m = (dst // shard) == c
        s = src[m]
        d = dst[m] - c * shard
        w = d >> 7
        b = s // BANK
        order = np.lexsort((w, b))
        s, d, w, b = s[order], d[order], w[order], b[order]
        np.add.at(cnt[c], (w, b), 1)
        per_core.append((s, d, w, b))

    G = _cdiv(cnt.max(axis=0), P)            # [nwin, nbank] groups (uniform)

    # global group (slot/msg) order: (bank, sw, window)
    # doff column order:            (bank, window)
    gslot = np.zeros((nwin, nbank), np.int64)   # group base in slot order
    gdoff = np.zeros((nwin, nbank), np.int64)   # doff col base of (w, b) run
    gops = []    # per (b, si): dict(g0, ng, ws=[w...])
    gpos = 0
    for b in range(nbank):
        for si in range(nsw):
            ws = list(range(si * SW, min((si + 1) * SW, nwin)))
            g0 = gpos
            for w in ws:
                gslot[w, b] = gpos
                gpos += G[w, b]
            if gpos > g0:
                gops.append(dict(b=b, g0=g0, ng=gpos - g0, ws=ws))
    GT = gpos
    dpos = 0
    for b in range(nbank):
        for w in range(nwin):
            gdoff[w, b] = dpos
            dpos += G[w, b]
    assert dpos == GT
    slots = GT * P

    idx16 = np.zeros((ncores, 128, slots // 16), np.int16)
    dstoff = np.empty((ncores, P, GT), np.float32)
    for c in range(ncores):
        s, d, w, b = per_core[c]
        gidx = np.zeros(slots, np.int64)         # bank-relative row (pad -> 0)
        wb = b * nwin + w
        runstart = np.concatenate([[0], 1 + np.flatnonzero(wb[1:] != wb[:-1])])
        rank = np.arange(len(wb)) - np.repeat(
            runstart, np.diff(np.concatenate([runstart, [len(wb)]])))
        slot = (gslot[w, b] + (rank >> 7)) * P + (rank & 127)
        gidx[slot] = s - b * BANK
        dslot = (gdoff[w, b] + (rank >> 7)) * P + (rank & 127)
        dofftmp = np.full(slots, -1.0, np.float32)
        dofftmp[dslot] = (d & 127).astype(np.float32)
        dstoff[c] = dofftmp.reshape(GT, P).T
        a = gidx.astype(np.int16).reshape(slots // 16, 16).T
        idx16[c] = np.tile(a, (8, 1))

    return dict(shard=shard, nwin=nwin, npad=npad, nbank=nbank, nsw=nsw,
                G=G, GT=GT, slots=slots, gops=gops,
                gslot=gslot, gdoff=gdoff,
                dinv=dinv, idx16=idx16, dstoff=dstoff)


# ----------------------------------------------------------------------------
# Device program
# ----------------------------------------------------------------------------
def _build(plan, N, CIN, ncores, unroll=1, skip=()):
    import concourse.bacc as bacc
    import concourse.tile as tile
    import concourse.mybir as mybir

    f32 = mybir.dt.float32
    bf16 = mybir.dt.bfloat16
    i16 = mybir.dt.int16
    nwin, npad, nbank = plan["nwin"], plan["npad"], plan["nbank"]
    G, GT, slots = plan["G"], plan["GT"], plan["slots"]
    gops, gslot, gdoff = plan["gops"], plan["gslot"], plan["gdoff"]
    padn = nwin * P
    kblk = CIN // P
    opGmax = max(op["ng"] for op in gops)
    wbGmax = int(G.max())

    nc = bacc.Bacc("TRN2", target_bir_lowering=False, debug=False,
                   enable_asserts=False, num_devices=ncores,
                   num_swdge_queues=NQ)

    xsT_in = nc.dram_tensor("xsT", [CIN, npad], bf16, kind="ExternalInput")
    w_in = nc.dram_tensor("w2", [P, kblk * COUT], bf16, kind="ExternalInput")
    idx_in = nc.dram_tensor("gidx", [P, slots // 16], i16,
                            kind="ExternalInput")
    doff_in = nc.dram_tensor("dstoff", [P, GT], bf16, kind="ExternalInput")
    iota_in = nc.dram_tensor("iota", [P, P], bf16, kind="ExternalInput")
    dinv_in = nc.dram_tensor("dinv_t", [P, nwin], f32, kind="ExternalInput")
    out_t = nc.dram_tensor("out", [padn, COUT], f32, kind="ExternalOutput")

    qi = 0

    with tile.TileContext(nc) as tc:
        with (
            tc.tile_pool(name="dram", bufs=1, space="DRAM") as dram,
            tc.tile_pool(name="const", bufs=1) as const,
            tc.tile_pool(name="acc", bufs=1) as accp,
            tc.tile_pool(name="xp", bufs=2) as xp,
            tc.tile_pool(name="hsb", bufs=3) as hsp,
            tc.tile_pool(name="idx", bufs=2) as idxp,
            tc.tile_pool(name="msgf", bufs=4) as msgfp,
            tc.tile_pool(name="seg", bufs=4) as segp,
            tc.tile_pool(name="osb", bufs=4) as osbp,
            tc.tile_pool(name="psA", bufs=2, space="PSUM") as psA,
            tc.tile_pool(name="psB", bufs=4, space="PSUM") as psB,
        ):
            h2b = [dram.tile([min(BANK, npad - b * BANK), COUT], f32,
                             name=f"h2b{b}")
                   for b in range(nbank)]

            for _it in range(unroll):
                w_sb = const.tile([P, kblk * COUT], bf16, tag="w_sb")
                nc.sync.dma_start(w_sb[:], w_in[:, :])
                iota_sb = const.tile([P, P], bf16, tag="iota")
                nc.sync.dma_start(iota_sb[:], iota_in[:, :])
                doff_sb = const.tile([P, GT], bf16, tag="doff")
                nc.sync.dma_start(doff_sb[:], doff_in[:, :])
                dinv_sb = const.tile([P, nwin], f32, tag="dinv")
                nc.sync.dma_start(dinv_sb[:], dinv_in[:, :])

                accs = []
                for w in range(nwin):
                    a_t = accp.tile([P, COUT], f32, tag=f"acc{w}",
                                    name=f"acc{w}")
                    nc.vector.memset(a_t[:], 0.0)
                    accs.append(a_t)

                # per-bank idx preloads on the ACT HWDGE ring (bypasses the
                # phase-A sync-engine DMA stream so bank-0 gathers can start
                # as soon as h bank 0 lands)
                bank_ops = [[op for op in gops if op["b"] == b]
                            for b in range(nbank)]
                bank_goff = [ops[0]["g0"] if ops else 0 for ops in bank_ops]
                bank_ng = [sum(op["ng"] for op in ops) for ops in bank_ops]
                bnGmax = max(bank_ng) if bank_ng else 0
                idx_sbs = {}
                for b in range(min(2, nbank)):
                    idx_sb = idxp.tile([P, bnGmax * 8], i16, tag="idx",
                                       name=f"idxb{b}")
                    nc.scalar.dma_start(
                        idx_sb[:, :bank_ng[b] * 8],
                        idx_in[:, bank_goff[b] * 8:
                               (bank_goff[b] + bank_ng[b]) * 8])
                    idx_sbs[b] = idx_sb

                # ---------------- Phase A: h = xs @ W (replicated, f32) ----
                for c0 in range(0, npad, CH):
                    xt = xp.tile([P, kblk, CH], bf16, tag="xt")
                    nc.sync.dma_start(
                        xt[:],
                        xsT_in[:, c0:c0 + CH].rearrange(
                            "(k p) n -> p k n", p=P))
                    hb = h2b[c0 // BANK]
                    r0 = c0 % BANK
                    for s in range(CH // 1024):
                        ps = psA.tile([P, 8, COUT], f32)
                        for t in range(8):
                            base = s * 1024 + t * P
                            for k in range(kblk):
                                nc.tensor.matmul(
                                    out=ps[:, t, :],
                                    lhsT=xt[:, k, base:base + P],
                                    rhs=w_sb[:, k * COUT:(k + 1) * COUT],
                                    start=(k == 0), stop=(k == kblk - 1))
                        hsb = hsp.tile([P, 8, COUT], f32, tag="hsb")
                        nc.scalar.copy(hsb[:], ps[:])
                        nc.sync.dma_start(
                            hb[r0 + s * 1024:r0 + (s + 1) * 1024, :]
                            .rearrange("(b p) q -> p b q", p=P),
                            hsb[:])

                # ---------------- Phase B: batched gather + one-hot matmul -
                if "phaseB" in skip:
                    continue
                for b in range(nbank):
                    idx_sb = idx_sbs.pop(b)
                    if b + 2 < nbank:   # prefetch next-next bank's idxs
                        nb = b + 2
                        nxt = idxp.tile([P, bnGmax * 8], i16, tag="idx",
                                        name=f"idxb{nb}")
                        nc.scalar.dma_start(
                            nxt[:, :bank_ng[nb] * 8],
                            idx_in[:, bank_goff[nb] * 8:
                                   (bank_goff[nb] + bank_ng[nb]) * 8])
                        idx_sbs[nb] = nxt
                    for op in bank_ops[b]:
                        g0, ng, ws = op["g0"], op["ng"], op["ws"]
                        l0 = g0 - bank_goff[b]
                        msgf = msgfp.tile([P, opGmax, COUT], f32, tag="msgf")
                        nc.gpsimd.dma_gather(
                            out_ap=msgf[:, :ng, :],
                            in_ap=h2b[b][:, :],
                            idxs_ap=idx_sb[:, l0 * 8:(l0 + ng) * 8],
                            num_idxs=ng * P,
                            num_idxs_reg=ng * P,
                            elem_size=COUT,
                            queue_num=qi % NQ,
                            single_packet=False,
                        )
                        qi += 1
                        for w in ws:
                            gw = int(G[w, b])
                            if gw == 0:
                                continue
                            d0 = int(gdoff[w, b])
                            m0 = int(gslot[w, b]) - g0
                            seg = segp.tile([P, wbGmax, P], bf16, tag="seg")
                            nc.vector.tensor_tensor(
                                out=seg[:, :gw, :],
                                in0=doff_sb[:, d0:d0 + gw, None]
                                    .to_broadcast([P, gw, P]),
                                in1=iota_sb[:, None, :]
                                    .to_broadcast([P, gw, P]),
                                op=mybir.AluOpType.is_equal)
                            ps = psB.tile([P, COUT], f32)
                            for j in range(gw):
                                rhs = (msgf[:, m0 + j, :]
                                       .bitcast(bf16)[:, 1::2])
                                nc.tensor.matmul(
                                    out=ps[:], lhsT=seg[:, j, :], rhs=rhs,
                                    start=(j == 0), stop=(j == gw - 1))
                            nc.vector.tensor_tensor(
                                out=accs[w][:], in0=ps[:], in1=accs[w][:],
                                op=mybir.AluOpType.add)

                for w in range(nwin):
                    o_sb = osbp.tile([P, COUT], f32, tag="osb")
                    nc.vector.tensor_scalar_mul(o_sb[:], accs[w][:],
                                                dinv_sb[:, w:w + 1])
                    nc.sync.dma_start(out_t[w * P:(w + 1) * P, :], o_sb[:])

    nc.compile()
    return nc


# ----------------------------------------------------------------------------
# Entry point
# ----------------------------------------------------------------------------
def _prepare(x, edge_index, W, b, ncores=8):
    from concourse import mybir

    bf16 = mybir.dt.np(mybir.dt.bfloat16)
    x = np.asarray(x)
    W = np.asarray(W)
    N, CIN = x.shape
    plan = _plan(edge_index, N, ncores)
    shard, nwin, npad = plan["shard"], plan["nwin"], plan["npad"]
    dinv = plan["dinv"]

    xsT = np.zeros((CIN, npad), dtype=bf16)
    xsT[:, :N] = (x.astype(np.float32) * dinv[:, None]).T.astype(bf16)
    w2 = np.concatenate([W[:P, :], W[P:, :]], axis=1).astype(bf16)
    iota = np.tile(np.arange(P, dtype=np.float32), (P, 1)).astype(bf16)

    in_maps = []
    for c in range(ncores):
        dv = np.zeros((nwin * P,), np.float32)
        dv[:shard] = dinv[c * shard:(c + 1) * shard]
        in_maps.append({
            "xsT": xsT,
            "w2": w2,
            "gidx": np.ascontiguousarray(plan["idx16"][c]),
            "dstoff": plan["dstoff"][c].astype(bf16),
            "iota": iota,
            "dinv_t": np.ascontiguousarray(dv.reshape(nwin, P).T),
        })
    return plan, in_maps


def kernel(x, edge_index, W, b, _trace=False):
    from concourse.bass_utils import run_bass_kernel_spmd

    x = np.asarray(x)
    W = np.asarray(W)
    b = np.asarray(b)
    N, CIN = x.shape
    ncores = 8
    plan, in_maps = _prepare(x, edge_index, W, b, ncores)
    shard = plan["shard"]

    nc = _build(plan, N, CIN, ncores)

    kernel.last_build = lambda unroll: (
        nc if unroll == 1 else _build(plan, N, CIN, ncores, unroll=unroll))
    kernel.last_in_maps = in_maps
    res = run_bass_kernel_spmd(nc, in_maps, core_ids=list(range(ncores)))
    out = np.concatenate(
        [r["out"][:shard].astype(np.float32) for r in res.results], axis=0)
    out = out + b.astype(np.float32)
    kernel.last_results = res
    return out


# revision 12
# speedup vs baseline: 1.8929x; 1.5854x over previous
"""GCNConv (gnn_message_passing) on 8 Trainium2 NeuronCores — v6.

out = D^{-1/2} (A + I) D^{-1/2} (X W) + b

Host folds dinv into x rows (prescale), so the device computes
  h = (dinv*x) @ W          (f32 table, replicated on every core)
  out[d] = dinv[d] * sum_{e: dst=d} h[src_e]     (self loops are edges)

Pipeline (v6): the h table lives in 4 per-bank DRAM tiles (32768 rows
each — the int16 reach of dma_gather).  Edge gathers are batched per
(bank, superwindow of SW dst-windows) into gpsimd.dma_gather ops
(~4.7k rows each, single_packet=False, 4 SWDGE queues round-robin);
bank-major order lets bank-b gathers start as soon as phase A has
written bank b, overlapping the dense phase under the gather stream.
Per-window partial sums accumulate across banks in an SBUF f32
accumulator: psum[128dst, 64] += seg^T @ msg per (window, bank), then
acc[:, w, :] += psum (DVE).  elem_size_bytes must be %256 -> f32 rows;
matmul rhs reads a stride-2 bf16 bitcast view (bf16 == high half of
f32), so no cast pass.  seg one-hots come from one is_equal per
(window, bank) on doff (laid out (bank, window)-major).  Final:
out[w] = acc * dinv, stored f32 node-major; host concat + bias.
"""

import numpy as np

P = 128
COUT = 64
CH = 4096          # phase A node chunk
NQ = 4             # SWDGE queues
BANK = 32768       # dma_gather int16 index reach (rows per bank)
SW = 4             # windows per superwindow (gather batch)


def _cdiv(a, b):
    return -(-a // b)


# ----------------------------------------------------------------------------
# CPU planning
# ----------------------------------------------------------------------------
def _plan(edge_index, N, ncores):
    shard = N // ncores
    nwin = _cdiv(shard, P)
    npad = _cdiv(N, CH) * CH if N % CH else N + CH   # h table rows
    nbank = _cdiv(npad, BANK)
    nsw = _cdiv(nwin, SW)

    src = np.asarray(edge_index[0], dtype=np.int64)
    dst = np.asarray(edge_index[1], dtype=np.int64)
    deg = np.bincount(dst, minlength=N).astype(np.float64) + 1.0
    dinv = (1.0 / np.sqrt(deg)).astype(np.float32)

    loop = np.arange(N, dtype=np.int64)
    src = np.concatenate([src, loop])
    dst = np.concatenate([dst, loop])

    per_core = []
    cnt = np.zeros((ncores, nwin, nbank), np.int64)
    for c in range(ncores):
        m = (dst // shard) == c
        s = src[m]
        d = dst[m] - c * shard
        w = d >> 7
        b = s // BANK
        order = np.lexsort((w, b))
        s, d, w, b = s[order], d[order], w[order], b[order]
        np.add.at(cnt[c], (w, b), 1)
        per_core.append((s, d, w, b))

    G = _cdiv(cnt.max(axis=0), P)            # [nwin, nbank] groups (uniform)

    # global group (slot/msg) order: (bank, sw, window)
    # doff column order:            (bank, window)
    gslot = np.zeros((nwin, nbank), np.int64)   # group base in slot order
    gdoff = np.zeros((nwin, nbank), np.int64)   # doff col base of (w, b) run
    gops = []    # per (b, si): dict(g0, ng, ws=[w...])
    gpos = 0
    for b in range(nbank):
        for si in range(nsw):
            ws = list(range(si * SW, min((si + 1) * SW, nwin)))
            g0 = gpos
            for w in ws:
                gslot[w, b] = gpos
                gpos += G[w, b]
            if gpos > g0:
                gops.append(dict(b=b, g0=g0, ng=gpos - g0, ws=ws))
    GT = gpos
    dpos = 0
    for b in range(nbank):
        for w in range(nwin):
            gdoff[w, b] = dpos
            dpos += G[w, b]
    assert dpos == GT
    slots = GT * P

    idx16 = np.zeros((ncores, 128, slots // 16), np.int16)
    dstoff = np.empty((ncores, P, GT), np.float32)
    for c in range(ncores):
        s, d, w, b = per_core[c]
        # pad slots must hit DISTINCT rows: same-row descriptor runs
        # serialize in the HBM path (measured 2x gather slowdown).
        # Spread them over the first 4096 rows of each bank (valid for
        # every bank; doff=-1 kills the gathered values).
        gidx = (np.arange(slots, dtype=np.int64) * 97) % 4096
        wb = b * nwin + w
        runstart = np.concatenate([[0], 1 + np.flatnonzero(wb[1:] != wb[:-1])])
        rank = np.arange(len(wb)) - np.repeat(
            runstart, np.diff(np.concatenate([runstart, [len(wb)]])))
        slot = (gslot[w, b] + (rank >> 7)) * P + (rank & 127)
        gidx[slot] = s - b * BANK
        dslot = (gdoff[w, b] + (rank >> 7)) * P + (rank & 127)
        dofftmp = np.full(slots, -1.0, np.float32)
        dofftmp[dslot] = (d & 127).astype(np.float32)
        dstoff[c] = dofftmp.reshape(GT, P).T
        a = gidx.astype(np.int16).reshape(slots // 16, 16).T
        idx16[c] = np.tile(a, (8, 1))

    return dict(shard=shard, nwin=nwin, npad=npad, nbank=nbank, nsw=nsw,
                G=G, GT=GT, slots=slots, gops=gops,
                gslot=gslot, gdoff=gdoff,
                dinv=dinv, idx16=idx16, dstoff=dstoff)


# ----------------------------------------------------------------------------
# Device program
# ----------------------------------------------------------------------------
def _build(plan, N, CIN, ncores, unroll=1, skip=()):
    import concourse.bacc as bacc
    import concourse.tile as tile
    import concourse.mybir as mybir

    f32 = mybir.dt.float32
    bf16 = mybir.dt.bfloat16
    i16 = mybir.dt.int16
    nwin, npad, nbank = plan["nwin"], plan["npad"], plan["nbank"]
    G, GT, slots = plan["G"], plan["GT"], plan["slots"]
    gops, gslot, gdoff = plan["gops"], plan["gslot"], plan["gdoff"]
    padn = nwin * P
    kblk = CIN // P
    opGmax = max(op["ng"] for op in gops)
    wbGmax = int(G.max())

    nc = bacc.Bacc("TRN2", target_bir_lowering=False, debug=False,
                   enable_asserts=False, num_devices=ncores,
                   num_swdge_queues=NQ)

    xsT_in = nc.dram_tensor("xsT", [CIN, npad], bf16, kind="ExternalInput")
    w_in = nc.dram_tensor("w2", [P, kblk * COUT], bf16, kind="ExternalInput")
    idx_in = nc.dram_tensor("gidx", [P, slots // 16], i16,
                            kind="ExternalInput")
    doff_in = nc.dram_tensor("dstoff", [P, GT], bf16, kind="ExternalInput")
    iota_in = nc.dram_tensor("iota", [P, P], bf16, kind="ExternalInput")
    dinv_in = nc.dram_tensor("dinv_t", [P, nwin], f32, kind="ExternalInput")
    out_t = nc.dram_tensor("out", [padn, COUT], f32, kind="ExternalOutput")

    qi = 0

    with tile.TileContext(nc) as tc:
        with (
            tc.tile_pool(name="dram", bufs=1, space="DRAM") as dram,
            tc.tile_pool(name="const", bufs=1) as const,
            tc.tile_pool(name="acc", bufs=1) as accp,
            tc.tile_pool(name="xp", bufs=2) as xp,
            tc.tile_pool(name="hsb", bufs=3) as hsp,
            tc.tile_pool(name="idx", bufs=2) as idxp,
            tc.tile_pool(name="msgf", bufs=4) as msgfp,
            tc.tile_pool(name="seg", bufs=4) as segp,
            tc.tile_pool(name="osb", bufs=4) as osbp,
            tc.tile_pool(name="psA", bufs=2, space="PSUM") as psA,
            tc.tile_pool(name="psB", bufs=4, space="PSUM") as psB,
        ):
            h2b = [dram.tile([min(BANK, npad - b * BANK), COUT], f32,
                             name=f"h2b{b}")
                   for b in range(nbank)]

            for _it in range(unroll):
                w_sb = const.tile([P, kblk * COUT], bf16, tag="w_sb")
                nc.sync.dma_start(w_sb[:], w_in[:, :])
                iota_sb = const.tile([P, P], bf16, tag="iota")
                nc.sync.dma_start(iota_sb[:], iota_in[:, :])
                doff_sb = const.tile([P, GT], bf16, tag="doff")
                nc.sync.dma_start(doff_sb[:], doff_in[:, :])
                dinv_sb = const.tile([P, nwin], f32, tag="dinv")
                nc.sync.dma_start(dinv_sb[:], dinv_in[:, :])

                accs = []
                for w in range(nwin):
                    a_t = accp.tile([P, COUT], f32, tag=f"acc{w}",
                                    name=f"acc{w}")
                    nc.vector.memset(a_t[:], 0.0)
                    accs.append(a_t)

                # per-bank idx preloads on the ACT HWDGE ring (bypasses the
                # phase-A sync-engine DMA stream so bank-0 gathers can start
                # as soon as h bank 0 lands)
                bank_ops = [[op for op in gops if op["b"] == b]
                            for b in range(nbank)]
                bank_goff = [ops[0]["g0"] if ops else 0 for ops in bank_ops]
                bank_ng = [sum(op["ng"] for op in ops) for ops in bank_ops]
                bnGmax = max(bank_ng) if bank_ng else 0
                idx_sbs = {}
                for b in range(min(2, nbank)):
                    idx_sb = idxp.tile([P, bnGmax * 8], i16, tag="idx",
                                       name=f"idxb{b}")
                    nc.scalar.dma_start(
                        idx_sb[:, :bank_ng[b] * 8],
                        idx_in[:, bank_goff[b] * 8:
                               (bank_goff[b] + bank_ng[b]) * 8])
                    idx_sbs[b] = idx_sb

                # ---------------- Phase A: h = xs @ W (replicated, f32) ----
                for c0 in ([] if "phaseA" in skip else range(0, npad, CH)):
                    xt = xp.tile([P, kblk, CH], bf16, tag="xt")
                    nc.sync.dma_start(
                        xt[:],
                        xsT_in[:, c0:c0 + CH].rearrange(
                            "(k p) n -> p k n", p=P))
                    hb = h2b[c0 // BANK]
                    r0 = c0 % BANK
                    for s in range(CH // 1024):
                        ps = psA.tile([P, 8, COUT], f32)
                        for t in range(8):
                            base = s * 1024 + t * P
                            for k in range(kblk):
                                nc.tensor.matmul(
                                    out=ps[:, t, :],
                                    lhsT=xt[:, k, base:base + P],
                                    rhs=w_sb[:, k * COUT:(k + 1) * COUT],
                                    start=(k == 0), stop=(k == kblk - 1))
                        hsb = hsp.tile([P, 8, COUT], f32, tag="hsb")
                        nc.scalar.copy(hsb[:], ps[:])
                        nc.sync.dma_start(
                            hb[r0 + s * 1024:r0 + (s + 1) * 1024, :]
                            .rearrange("(b p) q -> p b q", p=P),
                            hsb[:])

                # ---------------- Phase B: batched gather + one-hot matmul -
                if "phaseB" in skip:
                    continue
                for b in range(nbank):
                    idx_sb = idx_sbs.pop(b)
                    if b + 2 < nbank:   # prefetch next-next bank's idxs
                        nb = b + 2
                        nxt = idxp.tile([P, bnGmax * 8], i16, tag="idx",
                                        name=f"idxb{nb}")
                        nc.scalar.dma_start(
                            nxt[:, :bank_ng[nb] * 8],
                            idx_in[:, bank_goff[nb] * 8:
                                   (bank_goff[nb] + bank_ng[nb]) * 8])
                        idx_sbs[nb] = nxt
                    for op in bank_ops[b]:
                        g0, ng, ws = op["g0"], op["ng"], op["ws"]
                        l0 = g0 - bank_goff[b]
                        msgf = msgfp.tile([P, opGmax, COUT], f32, tag="msgf")
                        nc.gpsimd.dma_gather(
                            out_ap=msgf[:, :ng, :],
                            in_ap=h2b[b][:, :],
                            idxs_ap=idx_sb[:, l0 * 8:(l0 + ng) * 8],
                            num_idxs=ng * P,
                            num_idxs_reg=ng * P,
                            elem_size=COUT,
                            queue_num=qi % NQ,
                            single_packet=False,
                        )
                        qi += 1
                        if "consume" in skip:
                            dmy = osbp.tile([P, COUT], f32, tag="dmy",
                                            name="dmy")
                            nc.vector.tensor_copy(out=dmy[:],
                                                  in_=msgf[:, 0, :])
                            nc.sync.dma_start(
                                out_t[0:P, :], dmy[:])
                            continue
                        for w in ws:
                            gw = int(G[w, b])
                            if gw == 0:
                                continue
                            d0 = int(gdoff[w, b])
                            m0 = int(gslot[w, b]) - g0
                            seg = segp.tile([P, wbGmax, P], bf16, tag="seg")
                            nc.vector.tensor_tensor(
                                out=seg[:, :gw, :],
                                in0=doff_sb[:, d0:d0 + gw, None]
                                    .to_broadcast([P, gw, P]),
                                in1=iota_sb[:, None, :]
                                    .to_broadcast([P, gw, P]),
                                op=mybir.AluOpType.is_equal)
                            ps = psB.tile([P, COUT], f32)
                            for j in range(gw):
                                rhs = (msgf[:, m0 + j, :]
                                       .bitcast(bf16)[:, 1::2])
                                nc.tensor.matmul(
                                    out=ps[:], lhsT=seg[:, j, :], rhs=rhs,
                                    start=(j == 0), stop=(j == gw - 1))
                            nc.vector.tensor_tensor(
                                out=accs[w][:], in0=ps[:], in1=accs[w][:],
                                op=mybir.AluOpType.add)

                for w in (range(nwin) if "consume" not in skip else []):
                    o_sb = osbp.tile([P, COUT], f32, tag="osb")
                    nc.vector.tensor_scalar_mul(o_sb[:], accs[w][:],
                                                dinv_sb[:, w:w + 1])
                    nc.sync.dma_start(out_t[w * P:(w + 1) * P, :], o_sb[:])

    nc.compile()
    return nc


# ----------------------------------------------------------------------------
# Entry point
# ----------------------------------------------------------------------------
def _prepare(x, edge_index, W, b, ncores=8):
    from concourse import mybir

    bf16 = mybir.dt.np(mybir.dt.bfloat16)
    x = np.asarray(x)
    W = np.asarray(W)
    N, CIN = x.shape
    plan = _plan(edge_index, N, ncores)
    shard, nwin, npad = plan["shard"], plan["nwin"], plan["npad"]
    dinv = plan["dinv"]

    xsT = np.zeros((CIN, npad), dtype=bf16)
    xsT[:, :N] = (x.astype(np.float32) * dinv[:, None]).T.astype(bf16)
    w2 = np.concatenate([W[:P, :], W[P:, :]], axis=1).astype(bf16)
    iota = np.tile(np.arange(P, dtype=np.float32), (P, 1)).astype(bf16)

    in_maps = []
    for c in range(ncores):
        dv = np.zeros((nwin * P,), np.float32)
        dv[:shard] = dinv[c * shard:(c + 1) * shard]
        in_maps.append({
            "xsT": xsT,
            "w2": w2,
            "gidx": np.ascontiguousarray(plan["idx16"][c]),
            "dstoff": plan["dstoff"][c].astype(bf16),
            "iota": iota,
            "dinv_t": np.ascontiguousarray(dv.reshape(nwin, P).T),
        })
    return plan, in_maps


def kernel(x, edge_index, W, b, _trace=False):
    from concourse.bass_utils import run_bass_kernel_spmd

    x = np.asarray(x)
    W = np.asarray(W)
    b = np.asarray(b)
    N, CIN = x.shape
    ncores = 8
    plan, in_maps = _prepare(x, edge_index, W, b, ncores)
    shard = plan["shard"]

    nc = _build(plan, N, CIN, ncores)

    kernel.last_build = lambda unroll: (
        nc if unroll == 1 else _build(plan, N, CIN, ncores, unroll=unroll))
    kernel.last_in_maps = in_maps
    res = run_bass_kernel_spmd(nc, in_maps, core_ids=list(range(ncores)))
    out = np.concatenate(
        [r["out"][:shard].astype(np.float32) for r in res.results], axis=0)
    out = out + b.astype(np.float32)
    kernel.last_results = res
    return out
